# revision 26
# baseline (speedup 1.0000x reference)
"""Trainium2 Bass kernel for nn_ANEDecoder (Dia-style ANE decoder, 2 layers).

Sharding: tensor-parallel across 8 cores — 2 query heads + their (shared)
KV head per core for self-attn, 2 cross heads per core, FF/8 = 1024 MLP
hidden per core, all-reduce after o_proj / co_proj / wd.

Device layout: activations channels-first x[d, s] with the two batches
concatenated along the free dim (tiles [128, 256], col = b*128 + s).
Weights / KV caches are pre-cast to bf16 on the host (fp32 accumulation on
device); RMSNorm weights and attention scales are folded into projection
weights on the host.
"""
import os
import sys
import functools

sys.path.insert(0, "/opt/trn_rl_repo")

import numpy as np
import ml_dtypes

import concourse.bass as bass
import concourse.bacc as bacc
import concourse.mybir as mybir
import concourse.tile as tile
import concourse.masks as masks
from concourse.bass_utils import run_bass_kernel_spmd

BF = ml_dtypes.bfloat16
F32 = mybir.dt.float32
BF16 = mybir.dt.bfloat16
AF = mybir.ActivationFunctionType
ALU = mybir.AluOpType

# dims
B, D, S = 2, 2048, 128
A, T = 1536, 512
HQ, HKV, HD = 16, 4, 128
HC, HDC = 16, 128
FF, L = 8192, 2
EPS = 1e-5
NEG = -3e4

NCORES = 8
NDC = D // 128          # 16 d-chunks
SB = B * S              # 256, batch-concat free dim
QH = HQ // NCORES       # 2 query heads per core
CH = HC // NCORES       # 2 cross heads per core
FFS = FF // NCORES      # 1024 ff per core
NFC = FFS // 128        # 8 ff chunks
NAC = A // 128          # 12 cache chunks (self)
NTC = T // 128          # 4 cache chunks (cross)

_exec_time_ns = None


def last_exec_time_ns():
    return _exec_time_ns


# ---------------------------------------------------------------- builder
@functools.lru_cache(maxsize=4)
def build_graph(w: int):
    nc = bacc.Bacc()

    # ---- dram parameters (per-core shards arrive via in_maps) ----
    def par(name, shape, dt):
        return nc.declare_dram_parameter(name, list(shape), dt, isOutput=False)

    x_in = par("x_in", (D, SB), F32)
    qw = par("qw", (L, D, QH * HD), BF16)
    kw = par("kw", (L, D, HD), BF16)
    vw = par("vw", (L, D, HD), BF16)
    ow = par("ow", (L, QH * HD, D), BF16)
    cqw = par("cqw", (L, D, CH * HDC), BF16)
    cow = par("cow", (L, CH * HDC, D), BF16)
    wgw = par("wgw", (L, D, FFS), BF16)
    wuw = par("wuw", (L, D, FFS), BF16)
    wdw = par("wdw", (L, FFS, D), BF16)
    kT = par("kT", (L, B, HD, A), BF16)
    vT = par("vT", (L, B, A, HD), BF16)
    ckT = par("ckT", (L, B, CH, HDC, T), BF16)
    cvT = par("cvT", (L, B, CH, T, HDC), BF16)
    sinT = par("sinT", (HD // 2, SB), F32)
    cosT = par("cosT", (HD // 2, SB), F32)
    smask = par("smask", (S, B * A), BF16)
    cmask = par("cmask", (S, B * T), BF16)
    fnw = par("fnw", (128, NDC), F32)
    out_ext = nc.declare_dram_parameter("out", [D, SB], F32, isOutput=True)

    RG = [list(range(NCORES))]

    from contextlib import ExitStack

    with tile.TileContext(nc) as tc, ExitStack() as es:
        persist = es.enter_context(tc.tile_pool(name="persist", bufs=1))
        cachep = es.enter_context(tc.tile_pool(name="cache", bufs=2))
        wsmall = es.enter_context(tc.tile_pool(name="wsmall", bufs=6))
        wkv = es.enter_context(tc.tile_pool(name="wkv", bufs=6))
        wrow = es.enter_context(tc.tile_pool(name="wrow", bufs=2))
        wmlp = es.enter_context(tc.tile_pool(name="wmlp", bufs=4))
        wdp = es.enter_context(tc.tile_pool(name="wdp", bufs=8))
        hp = es.enter_context(tc.tile_pool(name="hp", bufs=18))
        sqp = es.enter_context(tc.tile_pool(name="sq", bufs=17))
        smallp = es.enter_context(tc.tile_pool(name="small", bufs=2))
        logitsp = es.enter_context(tc.tile_pool(name="logits", bufs=2))
        probsp = es.enter_context(tc.tile_pool(name="probs", bufs=2))
        ptp = es.enter_context(tc.tile_pool(name="ptp", bufs=4))
        attnp = es.enter_context(tc.tile_pool(name="attn", bufs=2))
        mtilep = es.enter_context(tc.tile_pool(name="mtile", bufs=2))
        dchp = es.enter_context(tc.tile_pool(name="dch", bufs=3))
        arpool = es.enter_context(tc.tile_pool(name="arp", bufs=3))
        outp = es.enter_context(tc.tile_pool(name="outp", bufs=2))
        pp = es.enter_context(tc.tile_pool(name="psum", bufs=8, space="PSUM"))
        dram = es.enter_context(tc.tile_pool(name="dram", bufs=1, space="DRAM"))
        if True:
            # ---------------- persistent tiles ----------------
            X = []
            for i in range(NDC):
                xt = persist.tile([128, SB], F32, name=f"X{i}", tag=f"X{i}")
                X.append(xt)
            ident = persist.tile([128, 128], BF16)
            masks.make_identity(nc, ident[:])
            ones_col = persist.tile([128, 1], BF16)       # partition-reduce lhsT
            nc.vector.memset(ones_col[:], 1.0)
            ones_row = persist.tile([1, 128], F32)        # partition-bcast lhsT
            nc.vector.memset(ones_row[:], 1.0)
            eps_t = persist.tile([1, 1], F32)
            nc.vector.memset(eps_t[:], EPS)
            sin_sb = persist.tile([64, SB], F32)
            cos_sb = persist.tile([64, SB], F32)
            smask_sb = persist.tile([S, B * A], BF16)
            cmask_sb = persist.tile([S, B * T], BF16)
            fnw_sb = persist.tile([128, NDC], F32)

            for i in range(NDC):
                nc.sync.dma_start(out=X[i][:], in_=x_in[i * 128:(i + 1) * 128, :])
            nc.sync.dma_start(out=sin_sb[:], in_=sinT[:])
            nc.sync.dma_start(out=cos_sb[:], in_=cosT[:])
            nc.sync.dma_start(out=smask_sb[:], in_=smask[:])
            nc.sync.dma_start(out=cmask_sb[:], in_=cmask[:])
            nc.sync.dma_start(out=fnw_sb[:], in_=fnw[:])

            # ---------------- collective warmup ----------------
            warm_src = smallp.tile([128, 16], BF16)
            nc.vector.memset(warm_src[:], 0.0)
            warm_in = dram.tile([128, 16], BF16)
            warm_out = dram.tile([128, 16], BF16)
            nc.sync.dma_start(out=warm_in[:], in_=warm_src[:])
            nc.gpsimd.collective_compute(
                "AllReduce", ALU.add, replica_groups=RG,
                ins=[warm_in.opt()], outs=[warm_out.opt()])
            warm_sb = smallp.tile([1, 16], F32)
            nc.gpsimd.dma_start(out=warm_sb[:], in_=warm_out[0:1, :])

            # AR bounce buffers (one pair per reduction point)
            ar_bufs = []
            for i in range(3 * L):
                ar_bufs.append((
                    dram.tile([D, SB], BF16, name=f"arin{i}", tag=f"arin{i}"),
                    dram.tile([D, SB], BF16, name=f"arout{i}", tag=f"arout{i}",
                              addr_space="Shared"),
                ))

            # ---------------- helpers ----------------
            def rmsnorm_h(tag):
                """Compute h = x * rsqrt(mean(x^2)+eps) in bf16 per d-chunk."""
                nstat = pp.tile([1, SB], F32, tag="psum")
                for i in range(NDC):
                    sq = sqp.tile([128, SB], BF16, tag="sq")
                    nc.scalar.activation(sq[:], X[i][:], AF.Square)
                    nc.tensor.matmul(nstat[:], ones_col[:], sq[:],
                                     start=(i == 0), stop=(i == NDC - 1))
                sd = smallp.tile([1, SB], F32, tag="sd")
                nc.scalar.activation(sd[:], nstat[:], AF.Sqrt,
                                     bias=eps_t[:], scale=1.0 / D)
                inv = smallp.tile([1, SB], F32, tag="inv")
                nc.vector.reciprocal(inv[:], sd[:])
                invb = pp.tile([128, SB], F32, tag="psum")
                nc.tensor.matmul(invb[:], ones_row[:], inv[:], start=True, stop=True)
                hs = []
                for i in range(NDC):
                    h = hp.tile([128, SB], BF16, tag="h")
                    nc.vector.tensor_mul(h[:], X[i][:], invb[:])
                    hs.append(h)
                return hs

            def rope_into(dst, dst_col, src_ap, src_col, b, width):
                """RoPE split-half rotary on a [128, width] column slice:
                reads src_ap[:, src_col:+width] (psum, fp32), writes bf16 into
                dst[:, dst_col:+width]; sin/cos at batch offset b*S."""
                t1 = smallp.tile([64, S], F32, tag="ropet", bufs=4)
                t2 = smallp.tile([64, S], F32, tag="ropet", bufs=4)
                sn = sin_sb[:, b * S:b * S + width]
                cs = cos_sb[:, b * S:b * S + width]
                x1 = src_ap[0:64, src_col:src_col + width]
                x2 = src_ap[64:128, src_col:src_col + width]
                nc.vector.tensor_mul(t1[:, :width], cs, x1)
                nc.vector.tensor_mul(t2[:, :width], sn, x2)
                nc.vector.tensor_sub(dst[0:64, dst_col:dst_col + width],
                                     t1[:, :width], t2[:, :width])
                t3 = smallp.tile([64, S], F32, tag="ropet", bufs=4)
                t4 = smallp.tile([64, S], F32, tag="ropet", bufs=4)
                nc.vector.tensor_mul(t3[:, :width], cs, x2)
                nc.vector.tensor_mul(t4[:, :width], sn, x1)
                nc.vector.tensor_add(dst[64:128, dst_col:dst_col + width],
                                     t3[:, :width], t4[:, :width])

            def allreduce_residual(slot, delta_sb):
                """delta_sb: assembled [128, NDC*SB] bf16 tile (d-chunk-major
                cols). One DMA to DRAM, AR, read back per chunk, add into X."""
                arin, arout = ar_bufs[slot]
                nc.sync.dma_start(
                    out=arin[:].rearrange("(c p) s -> p c s", p=128),
                    in_=delta_sb[:].rearrange("p (c s) -> p c s", s=SB))
                nc.gpsimd.collective_compute(
                    "AllReduce", ALU.add, replica_groups=RG,
                    ins=[arin.opt()], outs=[arout.opt()])
                for i in range(NDC):
                    art = arpool.tile([128, SB], BF16, tag="ar")
                    nc.sync.dma_start(out=art[:], in_=arout[i * 128:(i + 1) * 128, :])
                    arf = arpool.tile([128, SB], F32, tag="arf")
                    nc.vector.tensor_copy(arf[:], art[:])
                    nc.vector.tensor_add(X[i][:], X[i][:], arf[:])

            def second_proj(w_dram, l, n_e, delta_slot, act_sb):
                """delta[d, s] = sum_e W[e, d] act[e, s]; act_sb cols grouped
                (b, e-chunk): (b*n_e + e)*128 + s. Returns list of psum tiles
                (one per d-chunk pair, cols half*256)."""
                wts = []
                for ec in range(n_e):
                    wt = wrow.tile([128, D], BF16, tag="wrow")
                    nc.sync.dma_start(out=wt[:], in_=w_dram[l, ec * 128:(ec + 1) * 128, :])
                    wts.append(wt)
                delta_sb = dchp.tile([128, NDC * SB], BF16, tag="dsb", bufs=1)
                for dc in range(NDC):
                    pd = pp.tile([128, SB], F32, tag="psum")
                    for ec in range(n_e):
                        rhs = act_sb[:].rearrange(
                            "p (b e s) -> p e b s", b=B, e=n_e)[:, ec]
                        nc.tensor.matmul(
                            pd[:],
                            wts[ec][:, dc * 128:(dc + 1) * 128],
                            rhs, start=(ec == 0), stop=(ec == n_e - 1))
                    nc.scalar.activation(
                        delta_sb[:, dc * SB:(dc + 1) * SB], pd[:], AF.Copy)
                return delta_sb

            # ================= layers =================
            for l in range(L):
                # ---- prefetch KV caches for this layer (per-chunk tiles) ----
                kT_t, vT_t, ckT_t, cvT_t = {}, {}, {}, {}
                for b in range(B):
                    kt = cachep.tile([128, A], BF16, tag=f"kT{b}")
                    nc.sync.dma_start(out=kt[:], in_=kT[l, b])
                    kT_t[b] = kt
                    for cc in range(NAC):
                        vt = cachep.tile([128, 128], BF16, tag=f"vT{b}_{cc}")
                        nc.sync.dma_start(out=vt[:], in_=vT[l, b, cc * 128:(cc + 1) * 128, :])
                        vT_t[(b, cc)] = vt
                    for h in range(CH):
                        ck = cachep.tile([128, T], BF16, tag=f"ckT{b}_{h}")
                        nc.sync.dma_start(out=ck[:], in_=ckT[l, b, h])
                        ckT_t[(b, h)] = ck
                        for cc in range(NTC):
                            cv = cachep.tile([128, 128], BF16, tag=f"cvT{b}_{h}_{cc}")
                            nc.sync.dma_start(out=cv[:], in_=cvT[l, b, h, cc * 128:(cc + 1) * 128, :])
                            cvT_t[(b, h, cc)] = cv

                # ================ self-attention ================
                hs = rmsnorm_h("sa")

                # qkv projections: psum q0|q1 bank, k|v bank
                pq0 = pp.tile([128, SB], F32, tag="psum")
                pq1 = pp.tile([128, SB], F32, tag="psum")
                pk = pp.tile([128, SB], F32, tag="psum")
                pv = pp.tile([128, SB], F32, tag="psum")
                for i in range(NDC):
                    qwt = wsmall.tile([128, QH * HD], BF16, tag="wq")
                    nc.sync.dma_start(out=qwt[:], in_=qw[l, i * 128:(i + 1) * 128, :])
                    kwt = wkv.tile([128, HD], BF16, tag="wk")
                    nc.sync.dma_start(out=kwt[:], in_=kw[l, i * 128:(i + 1) * 128, :])
                    vwt = wkv.tile([128, HD], BF16, tag="wv")
                    nc.sync.dma_start(out=vwt[:], in_=vw[l, i * 128:(i + 1) * 128, :])
                    st, sp = (i == 0), (i == NDC - 1)
                    nc.tensor.matmul(pq0[:], qwt[:, 0:128], hs[i][:], start=st, stop=sp)
                    nc.tensor.matmul(pq1[:], qwt[:, 128:256], hs[i][:], start=st, stop=sp)
                    nc.tensor.matmul(pk[:], kwt[:], hs[i][:], start=st, stop=sp)
                    nc.tensor.matmul(pv[:], vwt[:], hs[i][:], start=st, stop=sp)

                # rope q into q_roped bf16 [128, (b h s)]
                q_roped = probsp.tile([128, B * QH * S], BF16, tag="qrope")
                for b in range(B):
                    for h in range(QH):
                        rope_into(q_roped, (b * QH + h) * S, (pq0 if h == 0 else pq1)[:],
                                  b * S, b, S)
                # rope k directly into kT cache columns [w, w+S) per batch
                for b in range(B):
                    rope_into(kT_t[b], w, pk[:], b * S, b, S)

                # v^T insert: transpose v [hd, s] -> [s, hd] per batch, write
                # into vT rows (w..w+S) == chunk/partition offsets
                vsb = probsp.tile([128, SB], BF16, tag="vsb")
                nc.scalar.activation(vsb[:], pv[:], AF.Copy)
                for b in range(B):
                    pvt = pp.tile([128, 128], BF16, tag="psum")
                    nc.tensor.transpose(pvt[:], vsb[:, b * S:(b + 1) * S], ident[:])
                    r = w % 128
                    c0 = w // 128
                    if r == 0:
                        nc.vector.tensor_copy(vT_t[(b, c0)][:], pvt[:])
                    else:
                        nc.vector.tensor_copy(vT_t[(b, c0)][r:128, :], pvt[0:128 - r, :])
                        nc.vector.tensor_copy(vT_t[(b, c0 + 1)][0:r, :], pvt[128 - r:128, :])

                # scores + softmax per (b, h)
                p_tiles = {}
                for b in range(B):
                    for h in range(QH):
                        logits = logitsp.tile([S, A], F32, tag="logits")
                        for ac in range(3):
                            ps = pp.tile([S, 512], F32, tag="psum")
                            nc.tensor.matmul(
                                ps[:],
                                q_roped[:, (b * QH + h) * S:(b * QH + h + 1) * S],
                                kT_t[b][:, ac * 512:(ac + 1) * 512],
                                start=True, stop=True)
                            nc.vector.tensor_add(
                                logits[:, ac * 512:(ac + 1) * 512], ps[:],
                                smask_sb[:, b * A + ac * 512:b * A + (ac + 1) * 512])
                        p = probsp.tile([S, A], BF16, tag="p", bufs=4)
                        denom = smallp.tile([S, 1], F32, tag="denom")
                        nc.scalar.activation(p[:], logits[:], AF.Exp,
                                             accum_out=denom[:])
                        invd = smallp.tile([S, 1], F32, tag="invd")
                        nc.vector.reciprocal(invd[:], denom[:])
                        nc.vector.tensor_scalar_mul(p[:], p[:], invd[:])
                        p_tiles[(b, h)] = p

                # attention: attn[hd, (b h s)] accumulated over cache chunks
                pattn = {}
                for b in range(B):
                    pattn[b] = pp.tile([128, QH * S], F32, tag="psum",
                                       name=f"pattn{l}_{b}")
                    for j in range(NAC):
                        pT = ptp.tile([128, QH * S], BF16, tag="pT")
                        for h in range(QH):
                            ptps = pp.tile([S, 128], BF16, tag="psum")
                            nc.tensor.transpose(
                                ptps[:], p_tiles[(b, h)][:, j * 128:(j + 1) * 128],
                                ident[:])
                            nc.vector.tensor_copy(pT[:, h * S:(h + 1) * S], ptps[:])
                        nc.tensor.matmul(
                            pattn[b][:],
                            vT_t[(b, j)][:],
                            pT[:], start=(j == 0), stop=(j == NAC - 1))
                attn_sb = attnp.tile([128, B * QH * S], BF16, tag="attn")
                for b in range(B):
                    nc.scalar.activation(
                        attn_sb[:, b * QH * S:(b + 1) * QH * S], pattn[b][:], AF.Copy)

                delta_sb = second_proj(ow, l, QH, 3 * l + 0, attn_sb)
                allreduce_residual(3 * l + 0, delta_sb)

                # ================ cross-attention ================
                hs = rmsnorm_h("ca")
                pcq0 = pp.tile([128, SB], F32, tag="psum")
                pcq1 = pp.tile([128, SB], F32, tag="psum")
                for i in range(NDC):
                    cqt = wsmall.tile([128, CH * HDC], BF16, tag="wcq")
                    nc.sync.dma_start(out=cqt[:], in_=cqw[l, i * 128:(i + 1) * 128, :])
                    st, sp = (i == 0), (i == NDC - 1)
                    nc.tensor.matmul(pcq0[:], cqt[:, 0:128], hs[i][:], start=st, stop=sp)
                    nc.tensor.matmul(pcq1[:], cqt[:, 128:256], hs[i][:], start=st, stop=sp)

                cq_roped = probsp.tile([128, B * CH * S], BF16, tag="qrope")
                for b in range(B):
                    for h in range(CH):
                        rope_into(cq_roped, (b * CH + h) * S, (pcq0 if h == 0 else pcq1)[:],
                                  b * S, b, S)

                cp_tiles = {}
                for b in range(B):
                    for h in range(CH):
                        ps = pp.tile([S, T], F32, tag="psum")
                        nc.tensor.matmul(
                            ps[:],
                            cq_roped[:, (b * CH + h) * S:(b * CH + h + 1) * S],
                            ckT_t[(b, h)][:],
                            start=True, stop=True)
                        clog = logitsp.tile([S, T], F32, tag="clogits")
                        nc.vector.tensor_add(clog[:], ps[:],
                                             cmask_sb[:, b * T:(b + 1) * T])
                        p = probsp.tile([S, T], BF16, tag="cp", bufs=4)
                        denom = smallp.tile([S, 1], F32, tag="denom")
                        nc.scalar.activation(p[:], clog[:], AF.Exp, accum_out=denom[:])
                        invd = smallp.tile([S, 1], F32, tag="invd")
                        nc.vector.reciprocal(invd[:], denom[:])
                        nc.vector.tensor_scalar_mul(p[:], p[:], invd[:])
                        cp_tiles[(b, h)] = p

                pcat = {}
                for b in range(B):
                    for h in range(CH):
                        pcat[(b, h)] = pp.tile([128, S], F32, tag="psum",
                                               name=f"pcat{l}_{b}_{h}")
                        for j in range(NTC):
                            pT = ptp.tile([S, 128], BF16, tag="cpT")
                            ptps = pp.tile([S, 128], BF16, tag="psum")
                            nc.tensor.transpose(
                                ptps[:], cp_tiles[(b, h)][:, j * 128:(j + 1) * 128],
                                ident[:])
                            nc.vector.tensor_copy(pT[:], ptps[:])
                            nc.tensor.matmul(
                                pcat[(b, h)][:],
                                cvT_t[(b, h, j)][:],
                                pT[:], start=(j == 0), stop=(j == NTC - 1))
                cattn_sb = attnp.tile([128, B * CH * S], BF16, tag="attn")
                for b in range(B):
                    for h in range(CH):
                        nc.scalar.activation(
                            cattn_sb[:, (b * CH + h) * S:(b * CH + h + 1) * S],
                            pcat[(b, h)][:], AF.Copy)

                delta_sb = second_proj(cow, l, CH, 3 * l + 1, cattn_sb)
                allreduce_residual(3 * l + 1, delta_sb)

                # ================ MLP ================
                hs = rmsnorm_h("mlp")
                m_tiles = []
                WW = 4  # fc-chunks per wave
                for wv in range(NFC // WW):
                    pgs, pus = [], []
                    for k in range(WW):
                        pgs.append(pp.tile([128, SB], F32, tag="psum",
                                           name=f"pg{l}_{wv}_{k}"))
                        pus.append(pp.tile([128, SB], F32, tag="psum",
                                           name=f"pu{l}_{wv}_{k}"))
                    for i in range(NDC):
                        g = wmlp.tile([128, WW * 128], BF16, tag="wg")
                        nc.sync.dma_start(
                            out=g[:], in_=wgw[l, i * 128:(i + 1) * 128,
                                             wv * WW * 128:(wv + 1) * WW * 128])
                        u = wmlp.tile([128, WW * 128], BF16, tag="wu")
                        nc.sync.dma_start(
                            out=u[:], in_=wuw[l, i * 128:(i + 1) * 128,
                                              wv * WW * 128:(wv + 1) * WW * 128])
                        st, sp = (i == 0), (i == NDC - 1)
                        for k in range(WW):
                            nc.tensor.matmul(pgs[k][:], g[:, k * 128:(k + 1) * 128],
                                             hs[i][:], start=st, stop=sp)
                            nc.tensor.matmul(pus[k][:], u[:, k * 128:(k + 1) * 128],
                                             hs[i][:], start=st, stop=sp)
                    for k in range(WW):
                        sg = mtilep.tile([128, SB], BF16, tag="sg")
                        nc.scalar.activation(sg[:], pgs[k][:], AF.Silu)
                        ub = mtilep.tile([128, SB], BF16, tag="ub")
                        nc.scalar.activation(ub[:], pus[k][:], AF.Copy)
                        m = mtilep.tile([128, SB], BF16, tag="m", bufs=8)
                        nc.vector.tensor_mul(m[:], sg[:], ub[:])
                        m_tiles.append(m)

                delta_sb = dchp.tile([128, NDC * SB], BF16, tag="dsb", bufs=1)
                for dhalf in range(2):
                    wd_sb = []
                    for fc in range(NFC):
                        t = wdp.tile([128, D // 2], BF16, tag="wd")
                        nc.sync.dma_start(
                            out=t[:],
                            in_=wdw[l, fc * 128:(fc + 1) * 128,
                                    dhalf * (D // 2):(dhalf + 1) * (D // 2)])
                        wd_sb.append(t)
                    for dc8 in range(NDC // 2):
                        dc = dhalf * (NDC // 2) + dc8
                        pd = pp.tile([128, SB], F32, tag="psum")
                        for fc in range(NFC):
                            nc.tensor.matmul(
                                pd[:],
                                wd_sb[fc][:, dc8 * 128:(dc8 + 1) * 128],
                                m_tiles[fc][:], start=(fc == 0), stop=(fc == NFC - 1))
                        nc.scalar.activation(
                            delta_sb[:, dc * SB:(dc + 1) * SB], pd[:], AF.Copy)
                allreduce_residual(3 * l + 2, delta_sb)

            # ================ final norm + output ================
            nstat = pp.tile([1, SB], F32, tag="psum")
            for i in range(NDC):
                sq = sqp.tile([128, SB], BF16, tag="sq")
                nc.scalar.activation(sq[:], X[i][:], AF.Square)
                nc.tensor.matmul(nstat[:], ones_col[:], sq[:],
                                 start=(i == 0), stop=(i == NDC - 1))
            sd = smallp.tile([1, SB], F32, tag="sd")
            nc.scalar.activation(sd[:], nstat[:], AF.Sqrt, bias=eps_t[:], scale=1.0 / D)
            inv = smallp.tile([1, SB], F32, tag="inv")
            nc.vector.reciprocal(inv[:], sd[:])
            invb = pp.tile([128, SB], F32, tag="psum")
            nc.tensor.matmul(invb[:], ones_row[:], inv[:], start=True, stop=True)
            # consume the warmup AR (zeros) so it isn't dead code
            nc.vector.tensor_scalar_mul(warm_sb[:], warm_sb[:], 0.0)
            for i in range(NDC):
                t = outp.tile([128, SB], F32, tag="outf")
                nc.vector.tensor_mul(t[:], X[i][:], invb[:])
                o = outp.tile([128, SB], F32, tag="outo")
                nc.scalar.activation(o[:], t[:], AF.Copy, scale=fnw_sb[:, i:i + 1])
                if i == 0:
                    nc.vector.tensor_add(o[0:1, 0:16], o[0:1, 0:16], warm_sb[:])
                nc.sync.dma_start(out=out_ext[i * 128:(i + 1) * 128, :], in_=o[:])

    nc.finalize()
    return nc


# ---------------------------------------------------------------- host prep
def _prep_in_maps(inputs):
    f32 = np.float32
    x = inputs["x"].astype(f32)                      # (B, D, 1, S)
    positions = inputs["positions"]
    w = int(np.asarray(inputs["kv_write_index"]).reshape(-1)[0])
    self_attn_mask = inputs["self_attn_mask"].astype(f32)  # (B,1,S,A)
    enc_len = np.asarray(inputs["encoder_lengths"]).reshape(B)

    sa_n = inputs["sa_norm_w"].astype(f32)[:, :, None]     # (L, D, 1)
    ca_n = inputs["ca_norm_w"].astype(f32)[:, :, None]
    mlp_n = inputs["mlp_norm_w"].astype(f32)[:, :, None]
    scale = 1.0 / np.sqrt(HD).astype(f32)
    cscale = 1.0 / np.sqrt(HDC).astype(f32)

    qw = (inputs["q_w"] * sa_n * scale).astype(BF)         # (L, D, HQ*HD)
    kw = (inputs["k_w"] * sa_n).astype(BF)
    vw = (inputs["v_w"] * sa_n).astype(BF)
    ow = inputs["o_w"].astype(BF)                          # (L, HQ*HD, D)
    cqw = (inputs["cq_w"] * ca_n * cscale).astype(BF)
    cow = inputs["co_w"].astype(BF)
    wgw = (inputs["wg_w"] * mlp_n).astype(BF)
    wuw = (inputs["wu_w"] * mlp_n).astype(BF)
    wdw = inputs["wd_w"].astype(BF)

    # caches, host-pretransposed: kT [L,B,HKV,HD,A], vT [L,B,HKV,A,HD]
    k_cache = inputs["k_cache"].reshape(L, B, HKV, A, HD)
    v_cache = inputs["v_cache"].reshape(L, B, HKV, HD, A)
    ck = inputs["ck_cache"].reshape(L, B, HC, T, HDC)
    cv = inputs["cv_cache"].reshape(L, B, HC, HDC, T)
    kTf = np.ascontiguousarray(k_cache.transpose(0, 1, 2, 4, 3)).astype(BF)
    vTf = np.ascontiguousarray(v_cache.transpose(0, 1, 2, 4, 3)).astype(BF)
    ckTf = np.ascontiguousarray(ck.transpose(0, 1, 2, 4, 3)).astype(BF)   # [L,B,HC,HDC,T]
    cvTf = np.ascontiguousarray(cv.transpose(0, 1, 2, 4, 3)).astype(BF)   # [L,B,HC,T,HDC]

    # rope tables [64, SB]
    inv_freq = 1.0 / (10000.0 ** (np.arange(0, HD, 2, dtype=f32) / HD))
    ang = positions.astype(f32)[:, None, :] * inv_freq[None, :, None]   # (B,64,S)
    sinT = np.ascontiguousarray(np.sin(ang).transpose(1, 0, 2).reshape(64, SB)).astype(f32)
    cosT = np.ascontiguousarray(np.cos(ang).transpose(1, 0, 2).reshape(64, SB)).astype(f32)

    smask = np.ascontiguousarray(
        self_attn_mask[:, 0].transpose(1, 0, 2).reshape(S, B * A)).astype(BF)
    t_idx = np.arange(T)
    cm = np.where(t_idx[None, :] < enc_len[:, None], 0.0, NEG).astype(f32)  # (B,T)
    cmask = np.broadcast_to(cm.reshape(1, B * T), (S, B * T)).astype(BF)
    cmask = np.ascontiguousarray(cmask)

    x_in = np.ascontiguousarray(
        x[:, :, 0, :].transpose(1, 0, 2).reshape(D, SB)).astype(f32)
    fnw = np.ascontiguousarray(
        inputs["final_norm_w"].astype(f32).reshape(NDC, 128).T)

    in_maps = []
    for c in range(NCORES):
        qh = slice(2 * c * HD, (2 * c + 2) * HD)
        kvh = c // 2
        ffs = slice(c * FFS, (c + 1) * FFS)
        in_maps.append({
            "x_in": x_in,
            "qw": np.ascontiguousarray(qw[:, :, qh]),
            "kw": np.ascontiguousarray(kw[:, :, kvh * HD:(kvh + 1) * HD]),
            "vw": np.ascontiguousarray(vw[:, :, kvh * HD:(kvh + 1) * HD]),
            "ow": np.ascontiguousarray(ow[:, qh, :]),
            "cqw": np.ascontiguousarray(cqw[:, :, qh]),
            "cow": np.ascontiguousarray(cow[:, qh, :]),
            "wgw": np.ascontiguousarray(wgw[:, :, ffs]),
            "wuw": np.ascontiguousarray(wuw[:, :, ffs]),
            "wdw": np.ascontiguousarray(wdw[:, ffs, :]),
            "kT": np.ascontiguousarray(kTf[:, :, kvh]),
            "vT": np.ascontiguousarray(vTf[:, :, kvh]),
            "ckT": np.ascontiguousarray(ckTf[:, :, 2 * c:2 * c + 2]),
            "cvT": np.ascontiguousarray(cvTf[:, :, 2 * c:2 * c + 2]),
            "sinT": sinT, "cosT": cosT,
            "smask": smask, "cmask": cmask,
            "fnw": fnw,
        })
    return in_maps, w


def kernel(**inputs):
    global _exec_time_ns
    in_maps, w = _prep_in_maps(inputs)
    nc = build_graph(w)
    trace = bool(int(os.environ.get("BASS_KERNEL_TRACE", "0")))
    res = run_bass_kernel_spmd(nc, in_maps, list(range(NCORES)), trace=trace)
    _exec_time_ns = res.exec_time_ns
    out = np.asarray(res.results[0]["out"])          # [D, SB] f32
    out = out.reshape(D, B, S).transpose(1, 0, 2)[:, :, None, :]
    return np.ascontiguousarray(out.astype(np.float32))
